# revision 16
# baseline (speedup 1.0000x reference)
"""CARAFE ghost-conv kernel for 8 Trainium2 NeuronCores (v2).

Self-contained: takes FULL inputs (as in setup_inputs()), returns FULL output
(4, 256, 160, 160) float32.

Sharding: 8 cores = 4 batches x 2 H-halves (data-parallel, halo'd on host).
Per core: input rows [40*half-6, 40*half+46) (52 rows, zero-padded outside the
image), W padded 80->84 (cols 2..81 valid). Output rows [80*half, 80*half+80).

v2 changes vs v1:
  - S2 down_cv2 dw5x5 moved DVE -> PE diag matmuls (kills the serial STT chain)
  - S5 softmax: k-sums on PE (Msum stationary), per-row DMA-XBAR transposes
    (PE transposes removed), packed-bf16 DVE normalize
  - S6 transposes -> DMA XBAR; zts shifted copies eliminated (stationary APs
    slice ztf at shifted partition bases with explicit tile_position)
  - CARAFE: block-diag over w in 32-col blocks stacked on partitions
    (dgS[80, 4, 9, 32], one 1152-el DVE build per rh instead of 2880) +
    27 PE-tile matmuls per rh via tile_position=(32b, 0)
  - outputs written bf16 (host converts to f32); single-pass SILU drain
  - S9: most superchunks on PE (diag), DVE_SCS superchunks on DVE STT chains
"""

import numpy as np
import ml_dtypes

import concourse.bacc as bacc
import concourse.bass as bass
import concourse.tile as tile
from concourse import mybir
from concourse.bass_utils import run_bass_kernel_spmd
from concourse.masks import make_identity

F32 = mybir.dt.float32
BF16 = mybir.dt.bfloat16
AF = mybir.ActivationFunctionType
ALU = mybir.AluOpType
AX = mybir.AxisListType

EPS = 1e-5
WP = 84          # padded low-res width
ROWS = 52        # local input rows (valid image rows at local 6..45)
NKT = 42         # kt / o rows (local rows 5..46)
NZ = 44          # Z rows kept (local rows 4..47)
WO = 164         # padded hi-res width
HO = 84          # hi-res rows (output rows 80*half-2 .. 80*half+82)

DVE_SCS = ()          # S9 superchunks computed on DVE (STT chains)
GPS_EVERY = 3         # every GPS_EVERY-th dgS build goes to gpsimd
PREF = 6              # dgS build prefetch depth (rh ahead of consumption)
TAPS_PER_RH = 2       # DVE S9 taps emitted per rh iteration


def _act(nc, out, in_, func, scale=1.0, bias=0.0):
    nc.scalar.activation(out=out, in_=in_, func=func, scale=scale, bias=bias)


def build_kernel():
    nc = bacc.Bacc("TRN2", target_bir_lowering=False, debug=False, num_devices=8)

    d = {}
    d["x_d"] = nc.declare_dram_parameter("x", [256, ROWS, WP], BF16, isOutput=False)
    d["edge_d"] = nc.declare_dram_parameter("edge", [128, 2], F32, isOutput=False)
    d["wdn1_d"] = nc.declare_dram_parameter("wdn1", [256, 32], BF16, isOutput=False)
    d["bdn1_d"] = nc.declare_dram_parameter("bdn1", [32, 2], F32, isOutput=False)
    d["ddn2c_d"] = nc.declare_dram_parameter("ddn2c", [128, 25], BF16, isOutput=False)
    d["bdn2_d"] = nc.declare_dram_parameter("bdn2", [128, 2], F32, isOutput=False)
    d["wencp_d"] = nc.declare_dram_parameter("wencp", [6, 128, 18], BF16, isOutput=False)
    d["benc1_d"] = nc.declare_dram_parameter("benc1", [18, 2], F32, isOutput=False)
    d["denc2c_d"] = nc.declare_dram_parameter("denc2c", [126, 25], BF16, isOutput=False)
    d["benc2_d"] = nc.declare_dram_parameter("benc2", [126, 2], F32, isOutput=False)
    d["wout1_d"] = nc.declare_dram_parameter("wout1", [256, 128], BF16, isOutput=False)
    d["bout1_d"] = nc.declare_dram_parameter("bout1", [128, 2], F32, isOutput=False)
    d["wto2_d"] = nc.declare_dram_parameter("wto2", [128, 25], F32, isOutput=False)
    d["bout2_d"] = nc.declare_dram_parameter("bout2", [128, 2], F32, isOutput=False)
    d["wmsum_d"] = nc.declare_dram_parameter("wmsum", [36, 4], BF16, isOutput=False)
    d["o0_d"] = nc.declare_dram_parameter("o0", [128, 80, 160], BF16, isOutput=True)
    d["o1_d"] = nc.declare_dram_parameter("o1", [128, 80, 160], BF16, isOutput=True)

    with tile.TileContext(nc) as tc:
        _emit(nc, tc, d)
    nc.compile()
    return nc


def _emit(nc, tc, d):
    x_d = d["x_d"]

    from contextlib import ExitStack
    ctx = ExitStack()
    with ctx:
        consts = ctx.enter_context(tc.tile_pool(name="consts", bufs=1))
        mid = ctx.enter_context(tc.tile_pool(name="mid", bufs=1))
        stage = ctx.enter_context(tc.tile_pool(name="stage", bufs=4))
        psA = ctx.enter_context(tc.tile_pool(name="psA", bufs=3, space="PSUM"))
        psT = ctx.enter_context(tc.tile_pool(name="psT", bufs=2, space="PSUM"))
        dgp = ctx.enter_context(tc.tile_pool(name="dgp", bufs=8))

        # ---- constants ---------------------------------------------------
        ident = consts.tile([128, 128], F32)
        make_identity(nc, ident[:])
        identb = consts.tile([128, 128], BF16)
        nc.vector.tensor_copy(identb[:], ident[:])
        identb32 = consts.tile([128, 32], BF16)
        for g in range(4):
            nc.sync.dma_start(out=identb32[32 * g:32 * g + 32, :],
                              in_=identb[0:32, 0:32])

        edge = consts.tile([128, 2], F32)
        nc.sync.dma_start(out=edge[:], in_=d["edge_d"][:])
        et, eb = edge[:, 0:1], edge[:, 1:2]

        wdn1 = consts.tile([128, 2, 32], BF16)
        nc.sync.dma_start(out=wdn1[:, 0, :], in_=d["wdn1_d"][0:128, :])
        nc.sync.dma_start(out=wdn1[:, 1, :], in_=d["wdn1_d"][128:256, :])
        bdn1 = consts.tile([32, 2], F32)
        nc.sync.dma_start(out=bdn1[:], in_=d["bdn1_d"][:])
        ddn2c = consts.tile([128, 25], BF16)
        nc.sync.dma_start(out=ddn2c[:], in_=d["ddn2c_d"][:])
        bdn2 = consts.tile([128, 2], F32)
        nc.sync.dma_start(out=bdn2[:], in_=d["bdn2_d"][:])
        wencp = consts.tile([128, 6, 18], BF16)
        nc.sync.dma_start(out=wencp[:], in_=d["wencp_d"][:].rearrange("t k m -> k t m"))
        benc1 = consts.tile([18, 2], F32)
        nc.sync.dma_start(out=benc1[:], in_=d["benc1_d"][:])
        denc2c = consts.tile([126, 25], BF16)
        nc.sync.dma_start(out=denc2c[:], in_=d["denc2c_d"][:])
        benc2 = consts.tile([126, 2], F32)
        nc.sync.dma_start(out=benc2[:], in_=d["benc2_d"][:])
        wout1 = consts.tile([128, 2, 128], BF16)
        nc.sync.dma_start(out=wout1[:, 0, :], in_=d["wout1_d"][0:128, :])
        nc.sync.dma_start(out=wout1[:, 1, :], in_=d["wout1_d"][128:256, :])
        bout1 = consts.tile([128, 2], F32)
        nc.sync.dma_start(out=bout1[:], in_=d["bout1_d"][:])
        wto2 = consts.tile([128, 25], F32)
        nc.sync.dma_start(out=wto2[:], in_=d["wto2_d"][:])
        bout2 = consts.tile([128, 2], F32)
        nc.sync.dma_start(out=bout2[:], in_=d["bout2_d"][:])
        wmsumb = consts.tile([36, 4], BF16)
        nc.sync.dma_start(out=wmsumb[:], in_=d["wmsum_d"][:])

        # diag stationaries (DVE builds; overlap the x input DMAs)
        dn2p = consts.tile([128, 25, 128], BF16)
        nc.vector.tensor_tensor(
            dn2p[:], identb[:].unsqueeze(1).to_broadcast((128, 25, 128)),
            ddn2c[:].unsqueeze(2).to_broadcast((128, 25, 128)), op=ALU.mult)
        denc2p = consts.tile([126, 25, 126], BF16)
        nc.vector.tensor_tensor(
            denc2p[:], identb[0:126, 0:126].unsqueeze(1)
                .to_broadcast((126, 25, 126)),
            denc2c[:].unsqueeze(2).to_broadcast((126, 25, 126)), op=ALU.mult)
        dout2 = consts.tile([128, 25, 128], BF16)
        with nc.allow_low_precision(reason="bf16 diag stationary build"):
            nc.vector.tensor_tensor(
                dout2[:], identb[:].unsqueeze(1).to_broadcast((128, 25, 128)),
                wto2[:].unsqueeze(2).to_broadcast((128, 25, 128)), op=ALU.mult)

        # mid-lived tensors (persist into the back phase)
        # zts[d][p, zr, c] = Z at img col p+d-2 (pre-shifted pixel-major
        # copies; stationary slices need 32-aligned partition bases)
        ztf = mid.tile([128, NZ, 128], BF16)
        zts = {d: mid.tile([128, NZ, 128], BF16, name=f"zts{d}")
               for d in (1, 2, 3)}
        ktn = mid.tile([80, NKT, 36], BF16)    # normalized kt (partition w = img col)
        e = mid.tile([36, ROWS, WP], F32)
        eexp48 = mid.tile([40, NKT, 80], BF16)

        with tc.tile_pool(name="early", bufs=1) as early:
            x0 = early.tile([128, ROWS, WP], BF16)
            x1 = early.tile([128, ROWS, WP], BF16)
            for i in range(4):
                r0, r1 = 13 * i, 13 * i + 13
                nc.sync.dma_start(out=x0[:, r0:r1, :], in_=x_d[0:128, r0:r1, :])
                nc.sync.dma_start(out=x1[:, r0:r1, :], in_=x_d[128:256, r0:r1, :])
            down_t = early.tile([128, ROWS * WP + 8], BF16)
            down = down_t[:, 4:4 + ROWS * WP].rearrange("p (r w) -> p r w", w=WP)
            e1bf = early.tile([18, ROWS, WP], BF16)
            zc = early.tile([128, NZ, WP], BF16)
            nc.gpsimd.memset(down_t[:, 0:4], 0.0)
            nc.gpsimd.memset(down_t[:, 4 + ROWS * WP:], 0.0)

            def win(flat, p0, p1, off, rr, w):
                return flat[p0:p1, off:off + rr * w].rearrange(
                    "p (r w) -> p r w", w=w)

            y1 = down[0:32]
            nc.vector.memset(down[32:64, 0:2, :], 0.0)
            nc.vector.memset(down[32:64, 50:52, :], 0.0)

            # ---- S1: down_cv1 + BN + SiLU -------------------------------
            for c0 in range(0, ROWS, 6):
                rr = min(6, ROWS - c0)
                ps = psA.tile([128, 6, WP], F32, tag="ps")
                nc.tensor.matmul(ps[0:32, 0:rr, :], wdn1[:, 0, :],
                                 x0[:, c0:c0 + rr, :], start=True, stop=False)
                nc.tensor.matmul(ps[0:32, 0:rr, :], wdn1[:, 1, :],
                                 x1[:, c0:c0 + rr, :], start=False, stop=True)
                _act(nc, y1[:, c0:c0 + rr, :], ps[0:32, 0:rr, :], AF.Silu,
                     scale=bdn1[:, 0:1], bias=bdn1[:, 1:2])
            nc.vector.tensor_scalar_mul(y1[:, 0:6, :], y1[:, 0:6, :], et[0:32])
            nc.vector.tensor_scalar_mul(y1[:, 46:52, :], y1[:, 46:52, :], eb[0:32])
            nc.vector.memset(y1[:, :, 0:2], 0.0)
            nc.vector.memset(y1[:, :, 82:84], 0.0)

            # ---- S6: Z = out_cv1 @ lo-res, pixel-major via DMA XBAR -----
            for c0 in range(0, NZ, 6):
                rr = min(6, NZ - c0)
                ps = psA.tile([128, 6, WP], F32, tag="ps")
                nc.tensor.matmul(ps[:, 0:rr, :], wout1[:, 0, :],
                                 x0[:, 4 + c0:4 + c0 + rr, :],
                                 start=True, stop=False)
                nc.tensor.matmul(ps[:, 0:rr, :], wout1[:, 1, :],
                                 x1[:, 4 + c0:4 + c0 + rr, :],
                                 start=False, stop=True)
                _act(nc, zc[:, c0:c0 + rr, 0:WP], ps[:, 0:rr, :], AF.Copy)
            for zr in range(NZ):
                pt6 = psT.tile([84, 128], BF16, tag="pt", name=f"pt6_{zr}")
                nc.tensor.transpose(pt6[:], zc[:, zr, 0:84], identb[:])
                nc.scalar.activation(out=ztf[0:84, zr, :], in_=pt6[:],
                                     func=AF.Copy)
            for dd in (1, 2, 3):
                for i in range(4):
                    r0, r1 = 11 * i, 11 * i + 11
                    nc.sync.dma_start(out=zts[dd][0:81, r0:r1, :],
                                      in_=ztf[dd:dd + 81, r0:r1, :])

            # ---- S2: down_cv2 (PE diag, 4 row-groups packed) ------------
            # group g (partitions 32g..32g+32) holds y1 rows [12g, 12g+16);
            # its outputs are rows [12g+2, 12g+14)
            y1s_t = early.tile([128, 16 * WP + 8], BF16)
            nc.gpsimd.memset(y1s_t[:, 0:4], 0.0)
            nc.gpsimd.memset(y1s_t[:, 4 + 16 * WP:], 0.0)
            for g in range(4):
                nc.sync.dma_start(
                    out=y1s_t[32 * g:32 * g + 32, 4:4 + 16 * WP],
                    in_=down_t[0:32, 4 + 12 * g * WP:4 + (12 * g + 16) * WP])
            for j in range(2):
                ps = psA.tile([128, 6, WP], F32, tag="ps")
                for t in range(25):
                    dh, dw = divmod(t, 5)
                    off = 4 + (6 * j + dh) * WP + (dw - 2)
                    nc.tensor.matmul(ps[:], dn2p[:, t, :],
                                     win(y1s_t, 0, 128, off, 6, WP),
                                     start=(t == 0), stop=(t == 24))
                st = stage.tile([128, 6, WP], BF16, tag="sty2")
                _act(nc, st[:], ps[:], AF.Silu,
                     scale=bdn2[:, 0:1], bias=bdn2[:, 1:2])
                # edge masks: rows 2..5 (group0 chunk0 rows 0..3, et),
                # rows 46..49 (group3 chunk1 rows 2..5, eb)
                if j == 0:
                    nc.vector.tensor_scalar_mul(st[0:32, 0:4, :], st[0:32, 0:4, :],
                                                et[0:32])
                else:
                    nc.vector.tensor_scalar_mul(st[96:128, 2:6, :], st[96:128, 2:6, :],
                                                eb[96:128])
                nc.vector.memset(st[:, :, 0:2], 0.0)
                nc.vector.memset(st[:, :, 82:84], 0.0)
                for g in range(4):
                    nc.sync.dma_start(
                        out=down[32:64, 12 * g + 2 + 6 * j:12 * g + 8 + 6 * j, :],
                        in_=st[32 * g:32 * g + 32, :, :])

            # shifted copy for S3 tap-pairing: partitions 64..127 hold the
            # 64-ch `down` features shifted by +1 flat element (4 chunks)
            q = ROWS * WP // 4
            for i in range(4):
                nc.sync.dma_start(
                    out=down_t[64:128, 4 + q * i:4 + q * (i + 1)],
                    in_=down_t[0:64, 5 + q * i:5 + q * (i + 1)])

            # ---- S3: enc_cv1 (9 taps, tap-paired) + BN + SiLU -----------
            S3_TAPS = (0, 3, 6, 2, 5, 8)
            for c0 in range(3, 49, 6):
                rr = min(6, 49 - c0)
                ps = psA.tile([128, 6, WP], F32, tag="ps")
                for i, t in enumerate(S3_TAPS):
                    dh, dw = divmod(t, 3)
                    off = 4 + (c0 - 1 + dh) * WP + (dw - 1)
                    nc.tensor.matmul(
                        ps[0:18, 0:rr, :], wencp[:, i, :],
                        win(down_t, 0, 128, off, rr, WP),
                        start=(i == 0), stop=(i == 5))
                _act(nc, e[0:18, c0:c0 + rr, :], ps[0:18, 0:rr, :], AF.Silu,
                     scale=benc1[:, 0:1], bias=benc1[:, 1:2])
            e1 = e[0:18]
            nc.vector.tensor_scalar_mul(e1[:, 3:6, :], e1[:, 3:6, :], et[0:18])
            nc.vector.tensor_scalar_mul(e1[:, 46:49, :], e1[:, 46:49, :], eb[0:18])
            nc.vector.memset(e1[:, :, 0:2], 0.0)
            nc.vector.memset(e1[:, :, 82:84], 0.0)
            nc.vector.memset(e1[:, 2:3, :], 0.0)
            nc.vector.memset(e1[:, 49:50, :], 0.0)
            nc.scalar.activation(out=e1bf[:, 2:50, :], in_=e1[:, 2:50, :],
                                 func=AF.Copy)

            # ---- S4: enc_cv2 (diag bf16, 7 row-groups packed) -----------
            # group g (partitions 18g..18g+18) holds e1 rows [6g+3, 6g+13);
            # outputs rows [6g+5, 6g+11)
            e1s_t = early.tile([128, 10 * WP + 8], BF16)
            nc.gpsimd.memset(e1s_t[:, 0:4], 0.0)
            nc.gpsimd.memset(e1s_t[:, 4 + 10 * WP:], 0.0)
            for g in range(7):
                nc.sync.dma_start(
                    out=e1s_t[18 * g:18 * g + 18, 4:4 + 10 * WP],
                    in_=e1bf[:, 6 * g + 3:6 * g + 13, :])
            ps = psA.tile([128, 6, WP], F32, tag="ps")
            for t in range(25):
                dh, dw = divmod(t, 5)
                off = 4 + dh * WP + (dw - 2)
                nc.tensor.matmul(
                    ps[0:126, 0:6, :], denc2p[:, t, :],
                    win(e1s_t, 0, 126, off, 6, WP),
                    start=(t == 0), stop=(t == 24))
            st = stage.tile([126, 6, WP], F32, tag="ste2")
            _act(nc, st[:], ps[0:126, :, :], AF.Silu,
                 scale=benc2[:, 0:1], bias=benc2[:, 1:2])
            for g in range(7):
                nc.sync.dma_start(out=e[18:36, 6 * g + 5:6 * g + 11, :],
                                  in_=st[18 * g:18 * g + 18, :, :])

        # early pool freed here
        from contextlib import ExitStack as _ES
        lctx = _ES()
        with lctx:
            late = lctx.enter_context(tc.tile_pool(name="late", bufs=1))
            ptpool = lctx.enter_context(tc.tile_pool(name="ptp", bufs=1,
                                                     space="PSUM"))
            accp = lctx.enter_context(tc.tile_pool(name="accp", bufs=2))
            o2bf_t = late.tile([128, HO * WO + 8], BF16)
            o2bf = o2bf_t[:, 4:4 + HO * WO].rearrange("p (r w) -> p r w", w=WO)
            nc.gpsimd.memset(o2bf_t[:, 0:4], 0.0)
            nc.gpsimd.memset(o2bf_t[:, 4 + HO * WO:], 0.0)
            nc.vector.memset(o2bf[:, :, 0:2], 0.0)
            nc.vector.memset(o2bf[:, :, 162:164], 0.0)

            # ---- S9 emission machinery ----------------------------------
            # PE superchunks split into 5-tap groups; DVE superchunks as
            # per-tap STT chains interleaved with the builds.
            s9_pe_groups = []       # queued closures, popped one per rh
            dve_taps = []           # queued (emit-closure) DVE taps
            dve_state = {}

            def queue_s9_pe(sc):
                g0 = 2 + 9 * sc
                gr = min(9, 82 - g0)
                nch = (gr + 2) // 3
                pss = [psA.tile([128, 3, 160], F32, tag="ps",
                                name=f"ps9_{sc}_{i}") for i in range(nch)]

                def group(t0, t1, sc=sc, g0=g0, gr=gr, nch=nch, pss=pss):
                    for t in range(t0, t1):
                        dh, dw = divmod(t, 5)
                        for ci in range(nch):
                            c0 = g0 + 3 * ci
                            rr = min(3, g0 + gr - c0)
                            nc.tensor.matmul(
                                pss[ci][:, 0:rr, :], dout2[:, t, :],
                                o2bf[:, c0 - 2 + dh:c0 - 2 + dh + rr,
                                     dw:dw + 160],
                                start=(t == 0), stop=(t == 24))
                    if t1 == 25:
                        st = stage.tile([128, 9, 160], BF16, tag="st9")
                        for ci in range(nch):
                            c0 = g0 + 3 * ci
                            rr = min(3, g0 + gr - c0)
                            _act(nc, st[:, 3 * ci:3 * ci + rr, :],
                                 pss[ci][:, 0:rr, :], AF.Silu,
                                 scale=bout2[:, 0:1], bias=bout2[:, 1:2])
                        nc.scalar.dma_start(
                            out=d["o1_d"][:, g0 - 2:g0 - 2 + gr, :],
                            in_=st[:, 0:gr, :])
                for i in range(5):
                    s9_pe_groups.append(lambda i=i: group(5 * i, 5 * i + 5))

            def queue_s9_dve(sc):
                g0 = 2 + 9 * sc
                gr = min(9, 82 - g0)
                acc = accp.tile([128, 9, 160], BF16, tag="acc9",
                                name=f"acc9_{sc}")

                def tap(t, sc=sc, g0=g0, gr=gr, acc=acc):
                    dh, dw = divmod(t, 5)
                    w = o2bf[:, g0 - 2 + dh:g0 - 2 + dh + gr, dw:dw + 160]
                    with nc.allow_low_precision(reason="bf16 dw-conv acc"):
                        if t == 0:
                            nc.vector.tensor_scalar_mul(
                                acc[:, 0:gr, :], w, wto2[:, 0:1])
                        else:
                            nc.vector.scalar_tensor_tensor(
                                out=acc[:, 0:gr, :], in0=w,
                                scalar=wto2[:, t:t + 1], in1=acc[:, 0:gr, :],
                                op0=ALU.mult, op1=ALU.add)
                    if t == 24:
                        st = stage.tile([128, 9, 160], BF16, tag="st9")
                        _act(nc, st[:, 0:gr, :], acc[:, 0:gr, :], AF.Silu,
                             scale=bout2[:, 0:1], bias=bout2[:, 1:2])
                        nc.scalar.dma_start(
                            out=d["o1_d"][:, g0 - 2:g0 - 2 + gr, :],
                            in_=st[:, 0:gr, :])
                for t in range(25):
                    dve_taps.append((sc, t, tap))

            def sc_ready(sc, rh):
                return rh >= (9 * sc + 14) // 2

            # dgS builds (emitted per-chunk right after normalize so DVE
            # reaches them early; consumed by the CARAFE loop below)
            dgs = {}

            def emit_build(rh):
                dg = dgp.tile([80, 9, 32, 4], BF16, tag="dg", name=f"dg_{rh}")
                eng = nc.gpsimd if rh % GPS_EVERY == GPS_EVERY - 1 else nc.vector
                eng.tensor_tensor(
                    dg[:],
                    identb32[0:80, :].unsqueeze(1).unsqueeze(3)
                        .to_broadcast((80, 9, 32, 4)),
                    ktn[0:80, rh, :].rearrange("p (k r) -> p k r", k=9)
                        .unsqueeze(2).to_broadcast((80, 9, 32, 4)),
                    op=ALU.mult)
                return dg


            state = {"sc": 0}

            def emit_back(rh):
                # release S9 superchunks whose input rows are complete
                while state["sc"] < 9 and sc_ready(state["sc"], rh):
                    if state["sc"] in DVE_SCS:
                        queue_s9_dve(state["sc"])
                    else:
                        queue_s9_pe(state["sc"])
                    state["sc"] += 1
                npop = 2 if len(s9_pe_groups) > 5 else 1
                for _ in range(min(npop, len(s9_pe_groups))):
                    s9_pe_groups.pop(0)()

                # ---- CARAFE: 27 tile matmuls (3 w-blocks x 9 taps) ------
                # each tile position accumulates into its OWN psum bank
                # (mixing positions within one bank fails walrus/hw)
                dg = dgs.pop(rh)
                pts = [ptpool.tile([128, 32 if b < 2 else 16, 4], F32,
                                   tag=f"ptp{b}", name=f"ptp{b}_{rh}")
                       for b in range(3)]
                for b in range(3):
                    wb = 32 if b < 2 else 16
                    for k in range(9):
                        dh, dw = divmod(k, 3)
                        nc.tensor.matmul(
                            pts[b][:],
                            zts[dw + 1][32 * b:32 * b + wb, rh + dh, :],
                            dg[32 * b:32 * b + wb, k, 0:wb, :],
                            start=(k == 0), stop=(k == 8),
                            tile_position=(32 * b, 0))

                # drain: BN+SiLU of out_cv1, interleave (w, r2) -> hi cols
                for b in range(3):
                    wb = 32 if b < 2 else 16
                    _act(nc, o2bf[:, 2 * rh:2 * rh + 2,
                                  2 + 64 * b:2 + 64 * b + 2 * wb]
                             .rearrange("p h (w q) -> p h w q", q=2),
                         pts[b][:].rearrange("p w (h q) -> p h w q", q=2),
                         AF.Silu, scale=bout1[:, 0:1], bias=bout1[:, 1:2])
                if rh == 0:
                    nc.vector.tensor_scalar_mul(
                        o2bf[:, 0:2, :], o2bf[:, 0:2, :], et)
                if rh == NKT - 1:
                    nc.vector.tensor_scalar_mul(
                        o2bf[:, 82:84, :], o2bf[:, 82:84, :], eb)
                # output channels 0..127 (valid rows only)
                if 1 <= rh <= 40:
                    nc.scalar.dma_start(
                        out=d["o0_d"][:, 2 * rh - 2:2 * rh, :],
                        in_=o2bf[:, 2 * rh:2 * rh + 2, 2:162])

                n = 0
                while dve_taps and n < TAPS_PER_RH:
                    sc, t, fn = dve_taps[0]
                    if not sc_ready(sc, rh):
                        break
                    dve_taps.pop(0)
                    fn(t)
                    n += 1

            # fused S5 + back-phase emission, chunk by chunk
            # ---- S5: softmax -> ktn[w, rh, (k r)] (bf16, pixel-major) ---
            # exp (scalar) -> per-(r,w) k-sums on PE -> recip (DVE f32) ->
            # bf16 recips appended -> per-rh XBAR transpose -> normalize
            for c0 in range(0, NKT, 6):
                _act(nc, eexp48[0:36, c0:c0 + 6, :],
                     e[0:36, 5 + c0:11 + c0, 2:82], AF.Exp)
                psk = psA.tile([4, 6, 80], F32, tag="ps")
                nc.tensor.matmul(psk[:], wmsumb[:],
                                 eexp48[0:36, c0:c0 + 6, :],
                                 start=True, stop=True)
                ssum = stage.tile([4, 6, 80], F32, tag="ssum")
                _act(nc, ssum[:], psk[:], AF.Copy)
                srow = stage.tile([4, 6, 80], BF16, tag="srow")
                with nc.allow_low_precision(reason="bf16 softmax recip"):
                    nc.vector.reciprocal(srow[:], ssum[:])
                nc.sync.dma_start(out=eexp48[36:40, c0:c0 + 6, :],
                                  in_=srow[:])
                for rh in range(c0, c0 + 6):
                    pt5 = psT.tile([84, 128], BF16, tag="pt",
                                   name=f"pt5_{rh}")
                    nc.tensor.transpose(pt5[0:80, 0:40],
                                        eexp48[0:40, rh, :],
                                        identb[0:40, 0:40])
                    ktt = stage.tile([80, 40], BF16, tag="ktt",
                                     name=f"ktt_{rh}")
                    nc.scalar.activation(out=ktt[:], in_=pt5[0:80, 0:40],
                                         func=AF.Copy)
                    nc.vector.tensor_tensor(
                        ktn[0:80, rh, :].rearrange("p (k r) -> p k r", k=9),
                        ktt[:, 0:36].rearrange("p (k r) -> p k r", k=9),
                        ktt[:, 36:40].unsqueeze(1)
                            .to_broadcast((80, 9, 4)),
                        op=ALU.mult)

                for rh in range(c0, c0 + 6):
                    dgs[rh] = emit_build(rh)
                for rh in range(c0, c0 + 6):
                    emit_back(rh)

            # flush remaining S9 work
            while state["sc"] < 9:
                if state["sc"] in DVE_SCS:
                    queue_s9_dve(state["sc"])
                else:
                    queue_s9_pe(state["sc"])
                state["sc"] += 1
            while s9_pe_groups:
                s9_pe_groups.pop(0)()
            while dve_taps:
                sc, t, fn = dve_taps.pop(0)
                fn(t)


# ---------------------------------------------------------------------------
# host side
# ---------------------------------------------------------------------------

_NC_CACHE = {}


def _get_nc():
    if "nc" not in _NC_CACHE:
        _NC_CACHE["nc"] = build_kernel()
    return _NC_CACHE["nc"]


def _bn2(g, b, m, v):
    inv = (g / np.sqrt(v + EPS)).astype(np.float32)
    beta = (b - m * inv).astype(np.float32)
    return np.stack([inv, beta], axis=1).astype(np.float32)


def _tile_bn(bn, rep):
    return np.tile(bn, (rep, 1))


def _wencp(w):
    # (18, 64, 3, 3) -> tap-paired stationaries (6, 128, 18); pair i covers
    # taps (t, t+1) through the +1-flat-shifted copy in partitions 64..127
    w9 = w.reshape(18, 64, 9).transpose(2, 1, 0)   # (9, 64, 18)
    out = np.zeros((6, 128, 18), np.float32)
    for i, t in enumerate((0, 3, 6, 2, 5, 8)):
        out[i, 0:64] = w9[t]
        if t in (0, 3, 6):
            out[i, 64:128] = w9[t + 1]
    return out.astype(ml_dtypes.bfloat16)


def prep_in_maps(inputs):
    inp = {k: np.asarray(v) for k, v in inputs.items()}
    x = inp["x"].astype(np.float32)

    wmsum = np.zeros((36, 4), np.float32)
    wmsum[np.arange(36), np.arange(36) % 4] = 1.0

    common = dict(
        wdn1=np.ascontiguousarray(inp["down_cv1_w"].reshape(32, 256).T).astype(ml_dtypes.bfloat16),
        bdn1=_bn2(inp["down_cv1_g"], inp["down_cv1_b"], inp["down_cv1_m"], inp["down_cv1_v"]),
        ddn2c=np.tile(inp["down_cv2_w"].reshape(32, 25), (4, 1)).astype(ml_dtypes.bfloat16),
        bdn2=_tile_bn(_bn2(inp["down_cv2_g"], inp["down_cv2_b"], inp["down_cv2_m"], inp["down_cv2_v"]), 4),
        wencp=_wencp(inp["enc_cv1_w"]),
        benc1=_bn2(inp["enc_cv1_g"], inp["enc_cv1_b"], inp["enc_cv1_m"], inp["enc_cv1_v"]),
        denc2c=np.tile(inp["enc_cv2_w"].reshape(18, 25), (7, 1)).astype(ml_dtypes.bfloat16),
        benc2=_tile_bn(_bn2(inp["enc_cv2_g"], inp["enc_cv2_b"], inp["enc_cv2_m"], inp["enc_cv2_v"]), 7),
        wout1=np.ascontiguousarray(inp["out_cv1_w"].reshape(128, 256).T).astype(ml_dtypes.bfloat16),
        bout1=_bn2(inp["out_cv1_g"], inp["out_cv1_b"], inp["out_cv1_m"], inp["out_cv1_v"]),
        wto2=np.ascontiguousarray(inp["out_cv2_w"].reshape(128, 25)).astype(np.float32),
        bout2=_bn2(inp["out_cv2_g"], inp["out_cv2_b"], inp["out_cv2_m"], inp["out_cv2_v"]),
        wmsum=wmsum.astype(ml_dtypes.bfloat16),
    )

    in_maps = []
    for s in range(8):
        n, half = s // 2, s % 2
        h0 = 40 * half
        xs = np.zeros((256, ROWS, WP), ml_dtypes.bfloat16)
        src_lo = max(0, h0 - 6)
        src_hi = min(80, h0 + 46)
        xs[:, src_lo - (h0 - 6):src_hi - (h0 - 6), 2:82] = x[n, :, src_lo:src_hi, :]
        edge = np.zeros((128, 2), np.float32)
        edge[:, 0] = 0.0 if half == 0 else 1.0
        edge[:, 1] = 1.0 if half == 0 else 0.0
        in_maps.append(dict(x=xs, edge=edge, **common))
    return in_maps


def kernel(**inputs):
    in_maps = prep_in_maps(inputs)
    nc = _get_nc()
    res = run_bass_kernel_spmd(nc, in_maps, list(range(8)))
    _NC_CACHE["last_result"] = res

    out = np.empty((4, 256, 160, 160), np.float32)
    for s in range(8):
        n, half = s // 2, s % 2
        r0, r1 = 80 * half, 80 * half + 80
        out[n, 0:128, r0:r1, :] = np.asarray(
            res.results[s]["o0"]).astype(np.float32)
        out[n, 128:256, r0:r1, :] = np.asarray(
            res.results[s]["o1"]).astype(np.float32)
    return out


# revision 17
# speedup vs baseline: 1.0935x; 1.0935x over previous
"""CARAFE ghost-conv kernel for 8 Trainium2 NeuronCores (v2).

Self-contained: takes FULL inputs (as in setup_inputs()), returns FULL output
(4, 256, 160, 160) float32.

Sharding: 8 cores = 4 batches x 2 H-halves (data-parallel, halo'd on host).
Per core: input rows [40*half-6, 40*half+46) (52 rows, zero-padded outside the
image), W padded 80->84 (cols 2..81 valid). Output rows [80*half, 80*half+80).

v2 changes vs v1:
  - S2 down_cv2 dw5x5 moved DVE -> PE diag matmuls (kills the serial STT chain)
  - S5 softmax: k-sums on PE (Msum stationary), per-row DMA-XBAR transposes
    (PE transposes removed), packed-bf16 DVE normalize
  - S6 transposes -> DMA XBAR; zts shifted copies eliminated (stationary APs
    slice ztf at shifted partition bases with explicit tile_position)
  - CARAFE: block-diag over w in 32-col blocks stacked on partitions
    (dgS[80, 4, 9, 32], one 1152-el DVE build per rh instead of 2880) +
    27 PE-tile matmuls per rh via tile_position=(32b, 0)
  - outputs written bf16 (host converts to f32); single-pass SILU drain
  - S9: most superchunks on PE (diag), DVE_SCS superchunks on DVE STT chains
"""

import numpy as np
import ml_dtypes

import concourse.bacc as bacc
import concourse.bass as bass
import concourse.tile as tile
from concourse import mybir
from concourse.bass_utils import run_bass_kernel_spmd
from concourse.masks import make_identity

F32 = mybir.dt.float32
BF16 = mybir.dt.bfloat16
AF = mybir.ActivationFunctionType
ALU = mybir.AluOpType
AX = mybir.AxisListType

EPS = 1e-5
WP = 84          # padded low-res width
ROWS = 52        # local input rows (valid image rows at local 6..45)
NKT = 42         # kt / o rows (local rows 5..46)
NZ = 44          # Z rows kept (local rows 4..47)
WO = 164         # padded hi-res width
HO = 84          # hi-res rows (output rows 80*half-2 .. 80*half+82)

DVE_SCS = ()          # S9 superchunks computed on DVE (STT chains)
GPS_EVERY = 3         # every GPS_EVERY-th dgS build goes to gpsimd
PREF = 6              # dgS build prefetch depth (rh ahead of consumption)
TAPS_PER_RH = 2       # DVE S9 taps emitted per rh iteration


def _act(nc, out, in_, func, scale=1.0, bias=0.0):
    nc.scalar.activation(out=out, in_=in_, func=func, scale=scale, bias=bias)


def build_kernel():
    nc = bacc.Bacc("TRN2", target_bir_lowering=False, debug=False, num_devices=8)

    d = {}
    d["x_d"] = nc.declare_dram_parameter("x", [256, ROWS, WP], BF16, isOutput=False)
    d["edge_d"] = nc.declare_dram_parameter("edge", [128, 2], F32, isOutput=False)
    d["wdn1_d"] = nc.declare_dram_parameter("wdn1", [256, 32], BF16, isOutput=False)
    d["bdn1_d"] = nc.declare_dram_parameter("bdn1", [32, 2], F32, isOutput=False)
    d["ddn2c_d"] = nc.declare_dram_parameter("ddn2c", [128, 25], BF16, isOutput=False)
    d["bdn2_d"] = nc.declare_dram_parameter("bdn2", [128, 2], F32, isOutput=False)
    d["wencp_d"] = nc.declare_dram_parameter("wencp", [6, 128, 18], BF16, isOutput=False)
    d["benc1_d"] = nc.declare_dram_parameter("benc1", [18, 2], F32, isOutput=False)
    d["denc2c_d"] = nc.declare_dram_parameter("denc2c", [126, 25], BF16, isOutput=False)
    d["benc2_d"] = nc.declare_dram_parameter("benc2", [126, 2], F32, isOutput=False)
    d["wout1_d"] = nc.declare_dram_parameter("wout1", [256, 128], BF16, isOutput=False)
    d["bout1_d"] = nc.declare_dram_parameter("bout1", [128, 2], F32, isOutput=False)
    d["wto2_d"] = nc.declare_dram_parameter("wto2", [128, 25], F32, isOutput=False)
    d["bout2_d"] = nc.declare_dram_parameter("bout2", [128, 2], F32, isOutput=False)
    d["wmsum_d"] = nc.declare_dram_parameter("wmsum", [36, 4], BF16, isOutput=False)
    d["o0_d"] = nc.declare_dram_parameter("o0", [128, 80, 160], BF16, isOutput=True)
    d["o1_d"] = nc.declare_dram_parameter("o1", [128, 80, 160], BF16, isOutput=True)

    with tile.TileContext(nc) as tc:
        _emit(nc, tc, d)
    nc.compile()
    return nc


def _emit(nc, tc, d):
    x_d = d["x_d"]

    from contextlib import ExitStack
    ctx = ExitStack()
    with ctx:
        consts = ctx.enter_context(tc.tile_pool(name="consts", bufs=1))
        mid = ctx.enter_context(tc.tile_pool(name="mid", bufs=1))
        stage = ctx.enter_context(tc.tile_pool(name="stage", bufs=4))
        psA = ctx.enter_context(tc.tile_pool(name="psA", bufs=4, space="PSUM"))
        dgp = ctx.enter_context(tc.tile_pool(name="dgp", bufs=8))

        # ---- constants ---------------------------------------------------
        ident = consts.tile([128, 128], F32)
        make_identity(nc, ident[:])
        identb = consts.tile([128, 128], BF16)
        nc.vector.tensor_copy(identb[:], ident[:])
        identb32 = consts.tile([128, 32], BF16)
        for g in range(4):
            nc.sync.dma_start(out=identb32[32 * g:32 * g + 32, :],
                              in_=identb[0:32, 0:32])

        edge = consts.tile([128, 2], F32)
        nc.sync.dma_start(out=edge[:], in_=d["edge_d"][:])
        et, eb = edge[:, 0:1], edge[:, 1:2]

        wdn1 = consts.tile([128, 2, 32], BF16)
        nc.sync.dma_start(out=wdn1[:, 0, :], in_=d["wdn1_d"][0:128, :])
        nc.sync.dma_start(out=wdn1[:, 1, :], in_=d["wdn1_d"][128:256, :])
        bdn1 = consts.tile([32, 2], F32)
        nc.sync.dma_start(out=bdn1[:], in_=d["bdn1_d"][:])
        ddn2c = consts.tile([128, 25], BF16)
        nc.sync.dma_start(out=ddn2c[:], in_=d["ddn2c_d"][:])
        bdn2 = consts.tile([128, 2], F32)
        nc.sync.dma_start(out=bdn2[:], in_=d["bdn2_d"][:])
        wencp = consts.tile([128, 6, 18], BF16)
        nc.sync.dma_start(out=wencp[:], in_=d["wencp_d"][:].rearrange("t k m -> k t m"))
        benc1 = consts.tile([18, 2], F32)
        nc.sync.dma_start(out=benc1[:], in_=d["benc1_d"][:])
        denc2c = consts.tile([126, 25], BF16)
        nc.sync.dma_start(out=denc2c[:], in_=d["denc2c_d"][:])
        benc2 = consts.tile([126, 2], F32)
        nc.sync.dma_start(out=benc2[:], in_=d["benc2_d"][:])
        wout1 = consts.tile([128, 2, 128], BF16)
        nc.sync.dma_start(out=wout1[:, 0, :], in_=d["wout1_d"][0:128, :])
        nc.sync.dma_start(out=wout1[:, 1, :], in_=d["wout1_d"][128:256, :])
        bout1 = consts.tile([128, 2], F32)
        nc.sync.dma_start(out=bout1[:], in_=d["bout1_d"][:])
        wto2 = consts.tile([128, 25], F32)
        nc.sync.dma_start(out=wto2[:], in_=d["wto2_d"][:])
        bout2 = consts.tile([128, 2], F32)
        nc.sync.dma_start(out=bout2[:], in_=d["bout2_d"][:])
        wmsumb = consts.tile([36, 4], BF16)
        nc.sync.dma_start(out=wmsumb[:], in_=d["wmsum_d"][:])

        # diag stationaries (DVE builds; overlap the x input DMAs)
        dn2p = consts.tile([128, 25, 128], BF16)
        nc.vector.tensor_tensor(
            dn2p[:], identb[:].unsqueeze(1).to_broadcast((128, 25, 128)),
            ddn2c[:].unsqueeze(2).to_broadcast((128, 25, 128)), op=ALU.mult)
        denc2p = consts.tile([126, 25, 126], BF16)
        nc.vector.tensor_tensor(
            denc2p[:], identb[0:126, 0:126].unsqueeze(1)
                .to_broadcast((126, 25, 126)),
            denc2c[:].unsqueeze(2).to_broadcast((126, 25, 126)), op=ALU.mult)
        dout2 = consts.tile([128, 25, 128], BF16)
        with nc.allow_low_precision(reason="bf16 diag stationary build"):
            nc.vector.tensor_tensor(
                dout2[:], identb[:].unsqueeze(1).to_broadcast((128, 25, 128)),
                wto2[:].unsqueeze(2).to_broadcast((128, 25, 128)), op=ALU.mult)

        # mid-lived tensors (persist into the back phase)
        # zts[d][p, zr, c] = Z at img col p+d-2 (pre-shifted pixel-major
        # copies; stationary slices need 32-aligned partition bases)
        ztf = mid.tile([128, NZ, 128], BF16)
        zts = {d: mid.tile([128, NZ, 128], BF16, name=f"zts{d}")
               for d in (1, 2, 3)}
        ktn = mid.tile([80, NKT, 36], BF16)    # normalized kt (partition w = img col)

        with tc.tile_pool(name="early", bufs=1) as early, \
                tc.tile_pool(name="psT", bufs=3, space="PSUM") as psT:
            x0 = early.tile([128, ROWS, WP], BF16)
            x1 = early.tile([128, ROWS, WP], BF16)
            for i in range(4):
                r0, r1 = 13 * i, 13 * i + 13
                nc.sync.dma_start(out=x0[:, r0:r1, :], in_=x_d[0:128, r0:r1, :])
                nc.sync.dma_start(out=x1[:, r0:r1, :], in_=x_d[128:256, r0:r1, :])
            down_t = early.tile([128, ROWS * WP + 8], BF16)
            down = down_t[:, 4:4 + ROWS * WP].rearrange("p (r w) -> p r w", w=WP)
            e = early.tile([36, ROWS, WP], F32)
            e1bf = early.tile([18, ROWS, WP], BF16)
            zc = early.tile([128, NZ, WP], BF16)
            eexp48 = early.tile([40, NKT, 80], BF16)
            nc.gpsimd.memset(down_t[:, 0:4], 0.0)
            nc.gpsimd.memset(down_t[:, 4 + ROWS * WP:], 0.0)

            def win(flat, p0, p1, off, rr, w):
                return flat[p0:p1, off:off + rr * w].rearrange(
                    "p (r w) -> p r w", w=w)

            y1 = down[0:32]
            nc.vector.memset(down[32:64, 0:2, :], 0.0)
            nc.vector.memset(down[32:64, 50:52, :], 0.0)

            # ---- S1: down_cv1 + BN + SiLU -------------------------------
            for c0 in range(0, ROWS, 6):
                rr = min(6, ROWS - c0)
                ps = psA.tile([128, 6, WP], F32, tag="ps")
                nc.tensor.matmul(ps[0:32, 0:rr, :], wdn1[:, 0, :],
                                 x0[:, c0:c0 + rr, :], start=True, stop=False)
                nc.tensor.matmul(ps[0:32, 0:rr, :], wdn1[:, 1, :],
                                 x1[:, c0:c0 + rr, :], start=False, stop=True)
                _act(nc, y1[:, c0:c0 + rr, :], ps[0:32, 0:rr, :], AF.Silu,
                     scale=bdn1[:, 0:1], bias=bdn1[:, 1:2])
            nc.vector.tensor_scalar_mul(y1[:, 0:6, :], y1[:, 0:6, :], et[0:32])
            nc.vector.tensor_scalar_mul(y1[:, 46:52, :], y1[:, 46:52, :], eb[0:32])
            nc.vector.memset(y1[:, :, 0:2], 0.0)
            nc.vector.memset(y1[:, :, 82:84], 0.0)

            # ---- S6: Z = out_cv1 @ lo-res, pixel-major via DMA XBAR -----
            for c0 in range(0, NZ, 6):
                rr = min(6, NZ - c0)
                ps = psA.tile([128, 6, WP], F32, tag="ps")
                nc.tensor.matmul(ps[:, 0:rr, :], wout1[:, 0, :],
                                 x0[:, 4 + c0:4 + c0 + rr, :],
                                 start=True, stop=False)
                nc.tensor.matmul(ps[:, 0:rr, :], wout1[:, 1, :],
                                 x1[:, 4 + c0:4 + c0 + rr, :],
                                 start=False, stop=True)
                _act(nc, zc[:, c0:c0 + rr, 0:WP], ps[:, 0:rr, :], AF.Copy)
            for zr in range(NZ):
                pt6 = psT.tile([84, 128], BF16, tag="pt", name=f"pt6_{zr}")
                nc.tensor.transpose(pt6[:], zc[:, zr, 0:84], identb[:])
                nc.scalar.activation(out=ztf[0:84, zr, :], in_=pt6[:],
                                     func=AF.Copy)
            for dd in (1, 2, 3):
                for i in range(4):
                    r0, r1 = 11 * i, 11 * i + 11
                    nc.sync.dma_start(out=zts[dd][0:81, r0:r1, :],
                                      in_=ztf[dd:dd + 81, r0:r1, :])

            # ---- S2: down_cv2 (PE diag, 4 row-groups packed) ------------
            # group g (partitions 32g..32g+32) holds y1 rows [12g, 12g+16);
            # its outputs are rows [12g+2, 12g+14)
            y1s_t = early.tile([128, 16 * WP + 8], BF16)
            nc.gpsimd.memset(y1s_t[:, 0:4], 0.0)
            nc.gpsimd.memset(y1s_t[:, 4 + 16 * WP:], 0.0)
            for g in range(4):
                nc.sync.dma_start(
                    out=y1s_t[32 * g:32 * g + 32, 4:4 + 16 * WP],
                    in_=down_t[0:32, 4 + 12 * g * WP:4 + (12 * g + 16) * WP])
            for j in range(2):
                ps = psA.tile([128, 6, WP], F32, tag="ps")
                for t in range(25):
                    dh, dw = divmod(t, 5)
                    off = 4 + (6 * j + dh) * WP + (dw - 2)
                    nc.tensor.matmul(ps[:], dn2p[:, t, :],
                                     win(y1s_t, 0, 128, off, 6, WP),
                                     start=(t == 0), stop=(t == 24))
                st = stage.tile([128, 6, WP], BF16, tag="sty2")
                _act(nc, st[:], ps[:], AF.Silu,
                     scale=bdn2[:, 0:1], bias=bdn2[:, 1:2])
                # edge masks: rows 2..5 (group0 chunk0 rows 0..3, et),
                # rows 46..49 (group3 chunk1 rows 2..5, eb)
                if j == 0:
                    nc.vector.tensor_scalar_mul(st[0:32, 0:4, :], st[0:32, 0:4, :],
                                                et[0:32])
                else:
                    nc.vector.tensor_scalar_mul(st[96:128, 2:6, :], st[96:128, 2:6, :],
                                                eb[96:128])
                nc.vector.memset(st[:, :, 0:2], 0.0)
                nc.vector.memset(st[:, :, 82:84], 0.0)
                for g in range(4):
                    nc.sync.dma_start(
                        out=down[32:64, 12 * g + 2 + 6 * j:12 * g + 8 + 6 * j, :],
                        in_=st[32 * g:32 * g + 32, :, :])

            # shifted copy for S3 tap-pairing: partitions 64..127 hold the
            # 64-ch `down` features shifted by +1 flat element (4 chunks)
            q = ROWS * WP // 4
            for i in range(4):
                nc.sync.dma_start(
                    out=down_t[64:128, 4 + q * i:4 + q * (i + 1)],
                    in_=down_t[0:64, 5 + q * i:5 + q * (i + 1)])

            # ---- S3: enc_cv1 (9 taps, tap-paired) + BN + SiLU -----------
            S3_TAPS = (0, 3, 6, 2, 5, 8)
            for c0 in range(3, 49, 6):
                rr = min(6, 49 - c0)
                ps = psA.tile([128, 6, WP], F32, tag="ps")
                for i, t in enumerate(S3_TAPS):
                    dh, dw = divmod(t, 3)
                    off = 4 + (c0 - 1 + dh) * WP + (dw - 1)
                    nc.tensor.matmul(
                        ps[0:18, 0:rr, :], wencp[:, i, :],
                        win(down_t, 0, 128, off, rr, WP),
                        start=(i == 0), stop=(i == 5))
                _act(nc, e[0:18, c0:c0 + rr, :], ps[0:18, 0:rr, :], AF.Silu,
                     scale=benc1[:, 0:1], bias=benc1[:, 1:2])
            e1 = e[0:18]
            nc.vector.tensor_scalar_mul(e1[:, 3:6, :], e1[:, 3:6, :], et[0:18])
            nc.vector.tensor_scalar_mul(e1[:, 46:49, :], e1[:, 46:49, :], eb[0:18])
            nc.vector.memset(e1[:, :, 0:2], 0.0)
            nc.vector.memset(e1[:, :, 82:84], 0.0)
            nc.vector.memset(e1[:, 2:3, :], 0.0)
            nc.vector.memset(e1[:, 49:50, :], 0.0)
            nc.scalar.activation(out=e1bf[:, 2:50, :], in_=e1[:, 2:50, :],
                                 func=AF.Copy)

            # ---- S4: enc_cv2 (diag bf16, 7 row-groups packed) -----------
            # group g (partitions 18g..18g+18) holds e1 rows [6g+3, 6g+13);
            # outputs rows [6g+5, 6g+11)
            e1s_t = early.tile([128, 10 * WP + 8], BF16)
            nc.gpsimd.memset(e1s_t[:, 0:4], 0.0)
            nc.gpsimd.memset(e1s_t[:, 4 + 10 * WP:], 0.0)
            for g in range(7):
                nc.sync.dma_start(
                    out=e1s_t[18 * g:18 * g + 18, 4:4 + 10 * WP],
                    in_=e1bf[:, 6 * g + 3:6 * g + 13, :])
            ps = psA.tile([128, 6, WP], F32, tag="ps")
            for t in range(25):
                dh, dw = divmod(t, 5)
                off = 4 + dh * WP + (dw - 2)
                nc.tensor.matmul(
                    ps[0:126, 0:6, :], denc2p[:, t, :],
                    win(e1s_t, 0, 126, off, 6, WP),
                    start=(t == 0), stop=(t == 24))
            st = stage.tile([126, 6, WP], F32, tag="ste2")
            _act(nc, st[:], ps[0:126, :, :], AF.Silu,
                 scale=benc2[:, 0:1], bias=benc2[:, 1:2])
            for g in range(7):
                nc.sync.dma_start(out=e[18:36, 6 * g + 5:6 * g + 11, :],
                                  in_=st[18 * g:18 * g + 18, :, :])

            # dgS builds (emitted per-chunk right after normalize so DVE
            # reaches them early; consumed by the CARAFE loop below)
            dgs = {}

            def emit_build(rh):
                dg = dgp.tile([80, 9, 32, 4], BF16, tag="dg", name=f"dg_{rh}")
                eng = nc.gpsimd if rh % GPS_EVERY == GPS_EVERY - 1 else nc.vector
                eng.tensor_tensor(
                    dg[:],
                    identb32[0:80, :].unsqueeze(1).unsqueeze(3)
                        .to_broadcast((80, 9, 32, 4)),
                    ktn[0:80, rh, :].rearrange("p (k r) -> p k r", k=9)
                        .unsqueeze(2).to_broadcast((80, 9, 32, 4)),
                    op=ALU.mult)
                return dg

            # ---- S5: softmax -> ktn[w, rh, (k r)] (bf16, pixel-major) ---
            # exp (scalar) -> per-(r,w) k-sums on PE -> recip (DVE f32) ->
            # bf16 recips appended -> per-rh XBAR transpose -> normalize
            for c0 in range(0, NKT, 6):
                _act(nc, eexp48[0:36, c0:c0 + 6, :],
                     e[0:36, 5 + c0:11 + c0, 2:82], AF.Exp)
                psk = psA.tile([4, 6, 80], F32, tag="ps")
                nc.tensor.matmul(psk[:], wmsumb[:],
                                 eexp48[0:36, c0:c0 + 6, :],
                                 start=True, stop=True)
                ssum = stage.tile([4, 6, 80], F32, tag="ssum")
                _act(nc, ssum[:], psk[:], AF.Copy)
                srow = stage.tile([4, 6, 80], BF16, tag="srow")
                with nc.allow_low_precision(reason="bf16 softmax recip"):
                    nc.vector.reciprocal(srow[:], ssum[:])
                nc.sync.dma_start(out=eexp48[36:40, c0:c0 + 6, :],
                                  in_=srow[:])
                for rh in range(c0, c0 + 6):
                    pt5 = psT.tile([84, 128], BF16, tag="pt",
                                   name=f"pt5_{rh}")
                    nc.tensor.transpose(pt5[0:80, 0:40],
                                        eexp48[0:40, rh, :],
                                        identb[0:40, 0:40])
                    ktt = stage.tile([80, 40], BF16, tag="ktt",
                                     name=f"ktt_{rh}")
                    nc.scalar.activation(out=ktt[:], in_=pt5[0:80, 0:40],
                                         func=AF.Copy)
                    nc.vector.tensor_tensor(
                        ktn[0:80, rh, :].rearrange("p (k r) -> p k r", k=9),
                        ktt[:, 0:36].rearrange("p (k r) -> p k r", k=9),
                        ktt[:, 36:40].unsqueeze(1)
                            .to_broadcast((80, 9, 4)),
                        op=ALU.mult)
                for rh in range(c0, c0 + 6):
                    dgs[rh] = emit_build(rh)

        # early pool freed here
        from contextlib import ExitStack as _ES
        lctx = _ES()
        with lctx:
            late = lctx.enter_context(tc.tile_pool(name="late", bufs=1))
            ptpool = lctx.enter_context(tc.tile_pool(name="ptp", bufs=1,
                                                     space="PSUM"))
            accp = lctx.enter_context(tc.tile_pool(name="accp", bufs=2))
            o2bf_t = late.tile([128, HO * WO + 8], BF16)
            o2bf = o2bf_t[:, 4:4 + HO * WO].rearrange("p (r w) -> p r w", w=WO)
            nc.gpsimd.memset(o2bf_t[:, 0:4], 0.0)
            nc.gpsimd.memset(o2bf_t[:, 4 + HO * WO:], 0.0)
            nc.vector.memset(o2bf[:, :, 0:2], 0.0)
            nc.vector.memset(o2bf[:, :, 162:164], 0.0)

            # ---- S9 emission machinery ----------------------------------
            # PE superchunks split into 5-tap groups; DVE superchunks as
            # per-tap STT chains interleaved with the builds.
            s9_pe_groups = []       # queued closures, popped one per rh
            dve_taps = []           # queued (emit-closure) DVE taps
            dve_state = {}

            def queue_s9_pe(sc):
                g0 = 2 + 9 * sc
                gr = min(9, 82 - g0)
                nch = (gr + 2) // 3
                pss = [psA.tile([128, 3, 160], F32, tag="ps",
                                name=f"ps9_{sc}_{i}") for i in range(nch)]

                def group(t0, t1, sc=sc, g0=g0, gr=gr, nch=nch, pss=pss):
                    for t in range(t0, t1):
                        dh, dw = divmod(t, 5)
                        for ci in range(nch):
                            c0 = g0 + 3 * ci
                            rr = min(3, g0 + gr - c0)
                            nc.tensor.matmul(
                                pss[ci][:, 0:rr, :], dout2[:, t, :],
                                o2bf[:, c0 - 2 + dh:c0 - 2 + dh + rr,
                                     dw:dw + 160],
                                start=(t == 0), stop=(t == 24))
                    if t1 == 25:
                        st = stage.tile([128, 9, 160], BF16, tag="st9")
                        for ci in range(nch):
                            c0 = g0 + 3 * ci
                            rr = min(3, g0 + gr - c0)
                            _act(nc, st[:, 3 * ci:3 * ci + rr, :],
                                 pss[ci][:, 0:rr, :], AF.Silu,
                                 scale=bout2[:, 0:1], bias=bout2[:, 1:2])
                        nc.scalar.dma_start(
                            out=d["o1_d"][:, g0 - 2:g0 - 2 + gr, :],
                            in_=st[:, 0:gr, :])
                for i in range(5):
                    s9_pe_groups.append(lambda i=i: group(5 * i, 5 * i + 5))

            def queue_s9_dve(sc):
                g0 = 2 + 9 * sc
                gr = min(9, 82 - g0)
                acc = accp.tile([128, 9, 160], BF16, tag="acc9",
                                name=f"acc9_{sc}")

                def tap(t, sc=sc, g0=g0, gr=gr, acc=acc):
                    dh, dw = divmod(t, 5)
                    w = o2bf[:, g0 - 2 + dh:g0 - 2 + dh + gr, dw:dw + 160]
                    with nc.allow_low_precision(reason="bf16 dw-conv acc"):
                        if t == 0:
                            nc.vector.tensor_scalar_mul(
                                acc[:, 0:gr, :], w, wto2[:, 0:1])
                        else:
                            nc.vector.scalar_tensor_tensor(
                                out=acc[:, 0:gr, :], in0=w,
                                scalar=wto2[:, t:t + 1], in1=acc[:, 0:gr, :],
                                op0=ALU.mult, op1=ALU.add)
                    if t == 24:
                        st = stage.tile([128, 9, 160], BF16, tag="st9")
                        _act(nc, st[:, 0:gr, :], acc[:, 0:gr, :], AF.Silu,
                             scale=bout2[:, 0:1], bias=bout2[:, 1:2])
                        nc.scalar.dma_start(
                            out=d["o1_d"][:, g0 - 2:g0 - 2 + gr, :],
                            in_=st[:, 0:gr, :])
                for t in range(25):
                    dve_taps.append((sc, t, tap))

            def sc_ready(sc, rh):
                return rh >= (9 * sc + 14) // 2

            next_sc = 0
            for rh in range(NKT):
                # release S9 superchunks whose input rows are complete
                while next_sc < 9 and sc_ready(next_sc, rh):
                    if next_sc in DVE_SCS:
                        queue_s9_dve(next_sc)
                    else:
                        queue_s9_pe(next_sc)
                    next_sc += 1
                npop = 2 if len(s9_pe_groups) > 5 else 1
                for _ in range(min(npop, len(s9_pe_groups))):
                    s9_pe_groups.pop(0)()

                # ---- CARAFE: 27 tile matmuls (3 w-blocks x 9 taps) ------
                # each tile position accumulates into its OWN psum bank
                # (mixing positions within one bank fails walrus/hw)
                dg = dgs.pop(rh)
                pts = [ptpool.tile([128, 32 if b < 2 else 16, 4], F32,
                                   tag=f"ptp{b}", name=f"ptp{b}_{rh}")
                       for b in range(3)]
                for b in range(3):
                    wb = 32 if b < 2 else 16
                    for k in range(9):
                        dh, dw = divmod(k, 3)
                        nc.tensor.matmul(
                            pts[b][:],
                            zts[dw + 1][32 * b:32 * b + wb, rh + dh, :],
                            dg[32 * b:32 * b + wb, k, 0:wb, :],
                            start=(k == 0), stop=(k == 8),
                            tile_position=(32 * b, 0))

                # drain: BN+SiLU of out_cv1, interleave (w, r2) -> hi cols
                for b in range(3):
                    wb = 32 if b < 2 else 16
                    _act(nc, o2bf[:, 2 * rh:2 * rh + 2,
                                  2 + 64 * b:2 + 64 * b + 2 * wb]
                             .rearrange("p h (w q) -> p h w q", q=2),
                         pts[b][:].rearrange("p w (h q) -> p h w q", q=2),
                         AF.Silu, scale=bout1[:, 0:1], bias=bout1[:, 1:2])
                if rh == 0:
                    nc.vector.tensor_scalar_mul(
                        o2bf[:, 0:2, :], o2bf[:, 0:2, :], et)
                if rh == NKT - 1:
                    nc.vector.tensor_scalar_mul(
                        o2bf[:, 82:84, :], o2bf[:, 82:84, :], eb)
                # output channels 0..127 (valid rows only)
                if 1 <= rh <= 40:
                    nc.scalar.dma_start(
                        out=d["o0_d"][:, 2 * rh - 2:2 * rh, :],
                        in_=o2bf[:, 2 * rh:2 * rh + 2, 2:162])

                n = 0
                while dve_taps and n < TAPS_PER_RH:
                    sc, t, fn = dve_taps[0]
                    if not sc_ready(sc, rh):
                        break
                    dve_taps.pop(0)
                    fn(t)
                    n += 1

            # flush remaining S9 work
            while next_sc < 9:
                if next_sc in DVE_SCS:
                    queue_s9_dve(next_sc)
                else:
                    queue_s9_pe(next_sc)
                next_sc += 1
            while s9_pe_groups:
                s9_pe_groups.pop(0)()
            while dve_taps:
                sc, t, fn = dve_taps.pop(0)
                fn(t)


# ---------------------------------------------------------------------------
# host side
# ---------------------------------------------------------------------------

_NC_CACHE = {}


def _get_nc():
    if "nc" not in _NC_CACHE:
        _NC_CACHE["nc"] = build_kernel()
    return _NC_CACHE["nc"]


def _bn2(g, b, m, v):
    inv = (g / np.sqrt(v + EPS)).astype(np.float32)
    beta = (b - m * inv).astype(np.float32)
    return np.stack([inv, beta], axis=1).astype(np.float32)


def _tile_bn(bn, rep):
    return np.tile(bn, (rep, 1))


def _wencp(w):
    # (18, 64, 3, 3) -> tap-paired stationaries (6, 128, 18); pair i covers
    # taps (t, t+1) through the +1-flat-shifted copy in partitions 64..127
    w9 = w.reshape(18, 64, 9).transpose(2, 1, 0)   # (9, 64, 18)
    out = np.zeros((6, 128, 18), np.float32)
    for i, t in enumerate((0, 3, 6, 2, 5, 8)):
        out[i, 0:64] = w9[t]
        if t in (0, 3, 6):
            out[i, 64:128] = w9[t + 1]
    return out.astype(ml_dtypes.bfloat16)


def prep_in_maps(inputs):
    inp = {k: np.asarray(v) for k, v in inputs.items()}
    x = inp["x"].astype(np.float32)

    wmsum = np.zeros((36, 4), np.float32)
    wmsum[np.arange(36), np.arange(36) % 4] = 1.0

    common = dict(
        wdn1=np.ascontiguousarray(inp["down_cv1_w"].reshape(32, 256).T).astype(ml_dtypes.bfloat16),
        bdn1=_bn2(inp["down_cv1_g"], inp["down_cv1_b"], inp["down_cv1_m"], inp["down_cv1_v"]),
        ddn2c=np.tile(inp["down_cv2_w"].reshape(32, 25), (4, 1)).astype(ml_dtypes.bfloat16),
        bdn2=_tile_bn(_bn2(inp["down_cv2_g"], inp["down_cv2_b"], inp["down_cv2_m"], inp["down_cv2_v"]), 4),
        wencp=_wencp(inp["enc_cv1_w"]),
        benc1=_bn2(inp["enc_cv1_g"], inp["enc_cv1_b"], inp["enc_cv1_m"], inp["enc_cv1_v"]),
        denc2c=np.tile(inp["enc_cv2_w"].reshape(18, 25), (7, 1)).astype(ml_dtypes.bfloat16),
        benc2=_tile_bn(_bn2(inp["enc_cv2_g"], inp["enc_cv2_b"], inp["enc_cv2_m"], inp["enc_cv2_v"]), 7),
        wout1=np.ascontiguousarray(inp["out_cv1_w"].reshape(128, 256).T).astype(ml_dtypes.bfloat16),
        bout1=_bn2(inp["out_cv1_g"], inp["out_cv1_b"], inp["out_cv1_m"], inp["out_cv1_v"]),
        wto2=np.ascontiguousarray(inp["out_cv2_w"].reshape(128, 25)).astype(np.float32),
        bout2=_bn2(inp["out_cv2_g"], inp["out_cv2_b"], inp["out_cv2_m"], inp["out_cv2_v"]),
        wmsum=wmsum.astype(ml_dtypes.bfloat16),
    )

    in_maps = []
    for s in range(8):
        n, half = s // 2, s % 2
        h0 = 40 * half
        xs = np.zeros((256, ROWS, WP), ml_dtypes.bfloat16)
        src_lo = max(0, h0 - 6)
        src_hi = min(80, h0 + 46)
        xs[:, src_lo - (h0 - 6):src_hi - (h0 - 6), 2:82] = x[n, :, src_lo:src_hi, :]
        edge = np.zeros((128, 2), np.float32)
        edge[:, 0] = 0.0 if half == 0 else 1.0
        edge[:, 1] = 1.0 if half == 0 else 0.0
        in_maps.append(dict(x=xs, edge=edge, **common))
    return in_maps


def kernel(**inputs):
    in_maps = prep_in_maps(inputs)
    nc = _get_nc()
    res = run_bass_kernel_spmd(nc, in_maps, list(range(8)))
    _NC_CACHE["last_result"] = res

    out = np.empty((4, 256, 160, 160), np.float32)
    for s in range(8):
        n, half = s // 2, s % 2
        r0, r1 = 80 * half, 80 * half + 80
        out[n, 0:128, r0:r1, :] = np.asarray(
            res.results[s]["o0"]).astype(np.float32)
        out[n, 128:256, r0:r1, :] = np.asarray(
            res.results[s]["o1"]).astype(np.float32)
    return out
